# revision 22
# baseline (speedup 1.0000x reference)
"""DeepSeekMoE forward on 8 Trainium2 NeuronCores (Bass/Tile, expert-parallel).

Per core c (SPMD, one program, per-core data):
  - routed experts 2c, 2c+1: the host computes top-2 routing bit-exactly with
    the reference's jax ops (eager, CPU), gathers each expert's tokens, pads
    to a common capacity C, and ships them transposed [D, 2C].  The device
    runs the expert MLP (fp32r matmuls, SiLU on ScalarE) and scales rows by
    the combine weight on PSUM eviction.
  - shared experts: NS-split — cores 0-3 run shared expert 0, cores 4-7 run
    shared expert 1, each over 1024 tokens (two 512-token MM1 sub-phases
    sharing one MM2 pass so shared weights stream once).
Host assembly: sum the two shared halves, scatter-add routed rows (indices
are unique within one expert).
"""
import sys

for _p in ("/opt/trn_rl_repo", "/root/.axon_site/_ro/trn_rl_repo"):
    if _p not in sys.path:
        sys.path.insert(0, _p)

import numpy as np

P = 128
B, S, D, H, E, NS = 2, 2048, 2048, 1408, 16, 2
T = B * S
TOP_K = 2
NCORES = 8
EPC = E // NCORES          # routed experts per core
TSH = T * NS // NCORES     # shared-expert tokens per core
SPH = 512                  # shared tokens per MM1 sub-phase
KC = D // P                # contraction chunks (16)
HC = H // P                # hidden chunks (11)
ND = 512                   # MM2 free-dim (d) tile
H_SLICES = (384, 384, 384, 256)        # w1 streaming slices (sum = H)
MM2_GROUP = 5              # concurrent MM2 PSUM accumulators

_NC_CACHE = {}


def _gating(xf, gate_w):
    """Bit-exact replica of the reference's gating math, eager on CPU."""
    import jax
    import jax.numpy as jnp

    cpu = jax.devices("cpu")[0]
    with jax.default_device(cpu):
        xj = jnp.asarray(xf)
        gj = jnp.asarray(gate_w)
        logits = jnp.einsum("td,ed->te", xj, gj)
        probs = jax.nn.softmax(logits, axis=-1)
        topk_w, topk_idx = jax.lax.top_k(probs, TOP_K)
        topk_wn = topk_w / jnp.sum(topk_w, axis=-1, keepdims=True)
        density_1 = jnp.mean(probs, axis=0)
        density_1_proxy = jnp.mean(logits, axis=0)
        aux = jnp.sum(density_1 * density_1_proxy) * E
    return (
        np.asarray(topk_wn, dtype=np.float32),
        np.asarray(topk_idx),
        np.asarray(aux, dtype=np.float32),
    )


def _halves(cap):
    """Split cap into x-load halves (multiples of P, streams >=256 when able)."""
    if cap <= 256:
        return [cap]
    h1 = -(-cap // 2 // P) * P
    return [h1, cap - h1]


def _pack_x(mat, caps):
    """[D, total] -> [P, KC*total]: per-half blocks [P, KC, ncols], flattened.

    caps: list of span widths; each span is packed per its _halves split.
    """
    total = mat.shape[1]
    out = np.empty((P, KC * total), dtype=np.float32)
    c0 = 0
    for cap in caps:
        for ncols in _halves(cap):
            blk = (
                mat[:, c0 : c0 + ncols]
                .reshape(KC, P, ncols)
                .transpose(1, 0, 2)
                .reshape(P, KC * ncols)
            )
            out[:, KC * c0 : KC * (c0 + ncols)] = blk
            c0 += ncols
    assert c0 == total
    return out


def _pack_w1(w):
    """[D, H] -> [P, KC*H]: per h-slice blocks [P, KC, hs], flattened."""
    out = np.empty((P, KC * H), dtype=np.float32)
    h0 = 0
    for hs in H_SLICES:
        blk = (
            w[:, h0 : h0 + hs]
            .reshape(KC, P, hs)
            .transpose(1, 0, 2)
            .reshape(P, KC * hs)
        )
        out[:, KC * h0 : KC * (h0 + hs)] = blk
        h0 += hs
    assert h0 == H
    return out


def _load_w1_slice(nc, w1pool, r32, w1_ap, h0, hs):
    w1t = w1pool.tile([P, KC, hs], r32, name="w1t", tag="w1t")
    nc.sync.dma_start(
        out=w1t[:],
        in_=w1_ap[:, KC * h0 : KC * (h0 + hs)].rearrange(
            "p (kc h) -> p kc h", kc=KC
        ),
    )
    return w1t


def _mm1_silu(nc, mybir, pools, x_loader, w1_ap, cap):
    """hT[h, t] = silu(sum_d w1[d, h] x[d, t]) for `cap` tokens."""
    f32 = mybir.dt.float32
    r32 = mybir.dt.float32r
    (xpool, w1pool, w2pool, hpool, cwpool, opool, psum1, psum2) = pools

    # first w1 slice before the x tiles: both gate the first matmul
    w1t = _load_w1_slice(nc, w1pool, r32, w1_ap, 0, H_SLICES[0])
    x_halves = x_loader()

    hT = hpool.tile([P, HC, cap], r32, name="hT", tag="hT")
    h0 = 0
    for si, hs in enumerate(H_SLICES):
        if si > 0:
            w1t = _load_w1_slice(nc, w1pool, r32, w1_ap, h0, hs)
        for hci in range(hs // P):
            hc = h0 // P + hci
            for (xt, c0, ncols) in x_halves:
                ps = psum1.tile([P, ncols], f32, name="ps1", tag="ps1")
                for kc in range(KC):
                    nc.tensor.matmul(
                        ps[:],
                        lhsT=w1t[:, kc, hci * P : (hci + 1) * P],
                        rhs=xt[:, kc, 0:ncols],
                        start=(kc == 0),
                        stop=(kc == KC - 1),
                    )
                nc.scalar.activation(
                    hT[:, hc, c0 : c0 + ncols], ps[:],
                    mybir.ActivationFunctionType.Silu,
                )
        h0 += hs
    assert h0 == H
    return hT


def _mm2_out(nc, mybir, pools, chunks, w2_ap):
    """y[t, d] = [cw[t] *] sum_h hT[h, t] w2[h, d].

    chunks: list of (hT, local_chunk, y_ap, y_row0, cwt_or_None, cw_col);
    streams w2 once per d0 across all chunks.
    """
    f32 = mybir.dt.float32
    r32 = mybir.dt.float32r
    (xpool, w1pool, w2pool, hpool, cwpool, opool, psum1, psum2) = pools

    for d0 in range(0, D, ND):
        w2_tiles = []
        for hc in range(HC):
            w2t = w2pool.tile([P, ND], r32, name="w2t", tag="w2t")
            nc.sync.dma_start(
                out=w2t[:], in_=w2_ap[hc * P : (hc + 1) * P, d0 : d0 + ND]
            )
            w2_tiles.append(w2t)
        for g0 in range(0, len(chunks), MM2_GROUP):
            group = chunks[g0 : g0 + MM2_GROUP]
            pss = [
                psum2.tile([P, ND], f32, name=f"ps2_{gi}", tag=f"ps2_{gi}")
                for gi in range(len(group))
            ]
            for hc in range(HC):
                for gi, (hT, lc, _, _, _, _) in enumerate(group):
                    nc.tensor.matmul(
                        pss[gi][:],
                        lhsT=hT[:, hc, lc * P : (lc + 1) * P],
                        rhs=w2_tiles[hc][:],
                        start=(hc == 0),
                        stop=(hc == HC - 1),
                    )
            for gi, (hT, lc, y_ap, y_row0, cwt, cw_col) in enumerate(group):
                ot = opool.tile([P, ND], f32, name="ot", tag="ot")
                if cwt is not None:
                    nc.vector.tensor_scalar_mul(
                        ot[:], pss[gi][:], cwt[:, cw_col : cw_col + 1]
                    )
                else:
                    nc.vector.tensor_copy(ot[:], pss[gi][:])
                r0 = y_row0 + lc * P
                nc.sync.dma_start(out=y_ap[r0 : r0 + P, d0 : d0 + ND], in_=ot[:])


def _build(caps):
    import concourse.tile as tile
    from concourse import bacc, mybir

    f32 = mybir.dt.float32
    r32 = mybir.dt.float32r
    CR = sum(caps)

    nc = bacc.Bacc("TRN2", target_bir_lowering=False, debug=False,
                   num_devices=NCORES)

    # pre-tiled layouts (host packs partition-major for long-contiguous DMA):
    #   xr_t/xs_t: per x-half blocks [P, KC, ncols], flattened to [P, free]
    #   wr1/sw1:   per h-slice blocks [P, KC, hs], flattened to [P, KC*H]
    xr_t = nc.declare_dram_parameter("xr_t", [P, KC * CR], r32,
                                     isOutput=False)
    cw = nc.declare_dram_parameter("cw", [P, CR // P], f32, isOutput=False)
    wr1 = nc.declare_dram_parameter("wr1", [EPC, P, KC * H], r32,
                                    isOutput=False)
    wr2 = nc.declare_dram_parameter("wr2", [EPC, H, D], r32, isOutput=False)
    xs_t = nc.declare_dram_parameter("xs_t", [P, KC * TSH], r32, isOutput=False)
    sw1 = nc.declare_dram_parameter("sw1", [P, KC * H], r32, isOutput=False)
    sw2 = nc.declare_dram_parameter("sw2", [H, D], r32, isOutput=False)
    yr = nc.declare_dram_parameter("yr", [CR, D], f32, isOutput=True)
    ys = nc.declare_dram_parameter("ys", [TSH, D], f32, isOutput=True)

    with tile.TileContext(nc) as tc:
        with (
            tc.tile_pool(name="xpool", bufs=3) as xpool,
            tc.tile_pool(name="w1pool", bufs=2) as w1pool,
            tc.tile_pool(name="w2pool", bufs=13) as w2pool,
            tc.tile_pool(name="hpool", bufs=2) as hpool,
            tc.tile_pool(name="cwpool", bufs=2) as cwpool,
            tc.tile_pool(name="opool", bufs=3) as opool,
            tc.tile_pool(name="psum1", bufs=3, space="PSUM") as psum1,
            tc.tile_pool(name="psum2", bufs=1, space="PSUM") as psum2,
        ):
            pools = (xpool, w1pool, w2pool, hpool, cwpool, opool, psum1, psum2)

            def load_x(x_ap, col0, cap):
                halves, c0 = [], 0
                for ncols in _halves(cap):
                    xt = xpool.tile([P, KC, ncols], r32, name="xt", tag="xt")
                    off = KC * (col0 + c0)
                    nc.sync.dma_start(
                        out=xt[:],
                        in_=x_ap[:, off : off + KC * ncols].rearrange(
                            "p (kc t) -> p kc t", kc=KC
                        ),
                    )
                    halves.append((xt, c0, ncols))
                    c0 += ncols
                return halves

            # ---- shared expert: two MM1 sub-phases, one fused MM2 ----
            sh_chunks = []
            for q in range(TSH // SPH):
                hTs = _mm1_silu(
                    nc, mybir, pools,
                    lambda q=q: load_x(xs_t[:, :], q * SPH, SPH),
                    sw1[:, :], SPH,
                )
                sh_chunks += [
                    (hTs, lc, ys, q * SPH, None, 0) for lc in range(SPH // P)
                ]
            _mm2_out(nc, mybir, pools, sh_chunks, sw2[:, :])

            # ---- routed experts ----
            col0 = 0
            for j, cap in enumerate(caps):
                tch = cap // P
                ch0 = col0 // P
                cwt = cwpool.tile([P, tch], f32, name="cwt", tag="cwt")
                nc.sync.dma_start(out=cwt[:], in_=cw[:, ch0 : ch0 + tch])
                hTr = _mm1_silu(
                    nc, mybir, pools,
                    lambda j=j, col0=col0, cap=cap: load_x(
                        xr_t[:, :], col0, cap
                    ),
                    wr1[j], cap,
                )
                chunks = [(hTr, lc, yr, col0, cwt, lc) for lc in range(tch)]
                _mm2_out(nc, mybir, pools, chunks, wr2[j])
                col0 += cap
    nc.compile()
    return nc


def _get_nc(caps):
    caps = tuple(caps)
    nc = _NC_CACHE.get(caps)
    if nc is None:
        nc = _build(caps)
        _NC_CACHE[caps] = nc
    return nc


def kernel(x, gate_w, shared_w1, shared_w2, w1, w2, _trace=False):
    from concourse.bass_utils import run_bass_kernel_spmd

    x = np.asarray(x, dtype=np.float32)
    gate_w = np.asarray(gate_w, dtype=np.float32)
    shared_w1 = np.ascontiguousarray(np.asarray(shared_w1, dtype=np.float32))
    shared_w2 = np.ascontiguousarray(np.asarray(shared_w2, dtype=np.float32))
    w1 = np.ascontiguousarray(np.asarray(w1, dtype=np.float32))
    w2 = np.ascontiguousarray(np.asarray(w2, dtype=np.float32))

    xf = x.reshape(T, D)
    topk_w, topk_idx, aux = _gating(xf, gate_w)

    # group token slots by expert
    flat_e = topk_idx.reshape(-1)
    flat_w = topk_w.reshape(-1).astype(np.float32)
    flat_t = np.repeat(np.arange(T, dtype=np.int64), TOP_K)
    order = np.argsort(flat_e, kind="stable")
    counts = np.bincount(flat_e, minlength=E)
    starts = np.zeros(E + 1, dtype=np.int64)
    np.cumsum(counts, out=starts[1:])

    tok_by_e = [flat_t[order[starts[e] : starts[e + 1]]] for e in range(E)]
    w_by_e = [flat_w[order[starts[e] : starts[e + 1]]] for e in range(E)]

    # slot j of every core serves the j-th NCORES-sized group of experts,
    # ranked by count, so each slot's capacity fits its group max
    rank = np.argsort(-counts, kind="stable")
    assign = [[int(rank[j * NCORES + c]) for j in range(EPC)]
              for c in range(NCORES)]
    caps = tuple(
        max(P, int(-(-max(counts[rank[j * NCORES + c]]
                          for c in range(NCORES)) // P)) * P)
        for j in range(EPC)
    )
    CR = sum(caps)
    col0s = [sum(caps[:j]) for j in range(EPC)]

    xfT = np.ascontiguousarray(xf.T)  # [D, T]

    sw1_packed = [_pack_w1(shared_w1[s]) for s in range(NS)]
    wr1_packed = [_pack_w1(w1[e]) for e in range(E)]

    in_maps = []
    for c in range(NCORES):
        xr_cols = np.zeros((D, CR), dtype=np.float32)
        cwv = np.zeros(CR, dtype=np.float32)
        for j in range(EPC):
            e = assign[c][j]
            cnt = int(counts[e])
            xr_cols[:, col0s[j] : col0s[j] + cnt] = xfT[:, tok_by_e[e]]
            cwv[col0s[j] : col0s[j] + cnt] = w_by_e[e]
        s = c // (NCORES // NS)
        q = c % (NCORES // NS)
        in_maps.append({
            "xr_t": _pack_x(xr_cols, caps),
            "cw": np.ascontiguousarray(cwv.reshape(CR // P, P).T),
            "wr1": np.stack([wr1_packed[assign[c][j]] for j in range(EPC)]),
            "wr2": np.stack([w2[assign[c][j]] for j in range(EPC)]),
            "xs_t": _pack_x(
                xfT[:, q * TSH : (q + 1) * TSH], [SPH] * (TSH // SPH)
            ),
            "sw1": sw1_packed[s],
            "sw2": shared_w2[s],
        })

    nc = _get_nc(caps)
    res = run_bass_kernel_spmd(nc, in_maps, core_ids=list(range(NCORES)),
                               trace=_trace)
    if _trace:
        kernel.last_results = res

    # assemble: shared halves summed, then routed scatter-adds
    out = np.empty((T, D), dtype=np.float32)
    for q in range(NCORES // NS):
        out[q * TSH : (q + 1) * TSH] = res.results[q]["ys"]
    for q in range(NCORES // NS):
        out[q * TSH : (q + 1) * TSH] += res.results[NCORES // NS + q]["ys"]
    for c in range(NCORES):
        for j in range(EPC):
            e = assign[c][j]
            cnt = int(counts[e])
            out[tok_by_e[e]] += res.results[c]["yr"][
                col0s[j] : col0s[j] + cnt
            ]

    return out.reshape(B, S, D), aux


# revision 25
# speedup vs baseline: 1.0007x; 1.0007x over previous
"""DeepSeekMoE forward on 8 Trainium2 NeuronCores (Bass/Tile, expert-parallel).

Per core c (SPMD, one program, per-core data):
  - routed experts 2c, 2c+1: the host computes top-2 routing bit-exactly with
    the reference's jax ops (eager, CPU), gathers each expert's tokens, pads
    to a common capacity C, and ships them transposed [D, 2C].  The device
    runs the expert MLP (fp32r matmuls, SiLU on ScalarE) and scales rows by
    the combine weight on PSUM eviction.
  - shared experts: NS-split — cores 0-3 run shared expert 0, cores 4-7 run
    shared expert 1, each over 1024 tokens (two 512-token MM1 sub-phases
    sharing one MM2 pass so shared weights stream once).
Host assembly: sum the two shared halves, scatter-add routed rows (indices
are unique within one expert).
"""
import sys

for _p in ("/opt/trn_rl_repo", "/root/.axon_site/_ro/trn_rl_repo"):
    if _p not in sys.path:
        sys.path.insert(0, _p)

import numpy as np

P = 128
B, S, D, H, E, NS = 2, 2048, 2048, 1408, 16, 2
T = B * S
TOP_K = 2
NCORES = 8
EPC = E // NCORES          # routed experts per core
TSH = T * NS // NCORES     # shared-expert tokens per core
SPH = 512                  # shared tokens per MM1 sub-phase
KC = D // P                # contraction chunks (16)
HC = H // P                # hidden chunks (11)
ND = 512                   # MM2 free-dim (d) tile
H_SLICES = (384, 384, 384, 256)        # w1 streaming slices (sum = H)
MM2_GROUP = 5              # concurrent MM2 PSUM accumulators

_NC_CACHE = {}


def _gating(xf, gate_w):
    """Bit-exact replica of the reference's gating math, eager on CPU."""
    import jax
    import jax.numpy as jnp

    cpu = jax.devices("cpu")[0]
    with jax.default_device(cpu):
        xj = jnp.asarray(xf)
        gj = jnp.asarray(gate_w)
        logits = jnp.einsum("td,ed->te", xj, gj)
        probs = jax.nn.softmax(logits, axis=-1)
        topk_w, topk_idx = jax.lax.top_k(probs, TOP_K)
        topk_wn = topk_w / jnp.sum(topk_w, axis=-1, keepdims=True)
        density_1 = jnp.mean(probs, axis=0)
        density_1_proxy = jnp.mean(logits, axis=0)
        aux = jnp.sum(density_1 * density_1_proxy) * E
    return (
        np.asarray(topk_wn, dtype=np.float32),
        np.asarray(topk_idx),
        np.asarray(aux, dtype=np.float32),
    )


def _halves(cap):
    """Split cap into x-load halves (multiples of P, streams >=256 when able)."""
    if cap <= 256:
        return [cap]
    h1 = -(-cap // 2 // P) * P
    return [h1, cap - h1]


def _pack_x(mat, caps):
    """[D, total] -> [P, KC*total]: per-half blocks [P, KC, ncols], flattened.

    caps: list of span widths; each span is packed per its _halves split.
    """
    total = mat.shape[1]
    out = np.empty((P, KC * total), dtype=np.float32)
    c0 = 0
    for cap in caps:
        for ncols in _halves(cap):
            blk = (
                mat[:, c0 : c0 + ncols]
                .reshape(KC, P, ncols)
                .transpose(1, 0, 2)
                .reshape(P, KC * ncols)
            )
            out[:, KC * c0 : KC * (c0 + ncols)] = blk
            c0 += ncols
    assert c0 == total
    return out


def _pack_w1(w):
    """[D, H] -> [P, KC*H]: per h-slice blocks [P, KC, hs], flattened."""
    out = np.empty((P, KC * H), dtype=np.float32)
    h0 = 0
    for hs in H_SLICES:
        blk = (
            w[:, h0 : h0 + hs]
            .reshape(KC, P, hs)
            .transpose(1, 0, 2)
            .reshape(P, KC * hs)
        )
        out[:, KC * h0 : KC * (h0 + hs)] = blk
        h0 += hs
    assert h0 == H
    return out


def _load_w1_slice(nc, w1pool, r32, w1_ap, h0, hs):
    w1t = w1pool.tile([P, KC, hs], r32, name="w1t", tag="w1t")
    nc.sync.dma_start(
        out=w1t[:],
        in_=w1_ap[:, KC * h0 : KC * (h0 + hs)].rearrange(
            "p (kc h) -> p kc h", kc=KC
        ),
    )
    return w1t


def _mm1_silu(nc, mybir, pools, x_loader, w1_ap, cap):
    """hT[h, t] = silu(sum_d w1[d, h] x[d, t]) for `cap` tokens."""
    f32 = mybir.dt.float32
    r32 = mybir.dt.float32r
    (xpool, w1pool, w2pool, hpool, cwpool, opool, psum1, psum2) = pools

    # first w1 slice before the x tiles: both gate the first matmul
    w1t = _load_w1_slice(nc, w1pool, r32, w1_ap, 0, H_SLICES[0])
    x_halves = x_loader()

    hT = hpool.tile([P, HC, cap], r32, name="hT", tag="hT")
    h0 = 0
    for si, hs in enumerate(H_SLICES):
        if si > 0:
            w1t = _load_w1_slice(nc, w1pool, r32, w1_ap, h0, hs)
        for hci in range(hs // P):
            hc = h0 // P + hci
            for (xt, c0, ncols) in x_halves:
                ps = psum1.tile([P, ncols], f32, name="ps1", tag="ps1")
                for kc in range(KC):
                    nc.tensor.matmul(
                        ps[:],
                        lhsT=w1t[:, kc, hci * P : (hci + 1) * P],
                        rhs=xt[:, kc, 0:ncols],
                        start=(kc == 0),
                        stop=(kc == KC - 1),
                    )
                nc.scalar.activation(
                    hT[:, hc, c0 : c0 + ncols], ps[:],
                    mybir.ActivationFunctionType.Silu,
                )
        h0 += hs
    assert h0 == H
    return hT


def _group_splits(n, tail_single):
    splits = list(range(0, n, MM2_GROUP))
    if tail_single and n > 1 and (n - splits[-1]) > 1:
        splits.append(n - 1)
    return splits + [n]


def _mm2_out(nc, mybir, pools, chunks, w2_ap, tail_single=False):
    """y[t, d] = [cw[t] *] sum_h hT[h, t] w2[h, d].

    chunks: list of (hT, local_chunk, y_ap, y_row0, cwt_or_None, cw_col);
    streams w2 once per d0 across all chunks.
    """
    f32 = mybir.dt.float32
    r32 = mybir.dt.float32r
    (xpool, w1pool, w2pool, hpool, cwpool, opool, psum1, psum2) = pools

    for d0 in range(0, D, ND):
        w2_tiles = []
        for hc in range(HC):
            w2t = w2pool.tile([P, ND], r32, name="w2t", tag="w2t")
            nc.sync.dma_start(
                out=w2t[:], in_=w2_ap[hc * P : (hc + 1) * P, d0 : d0 + ND]
            )
            w2_tiles.append(w2t)
        splits = _group_splits(len(chunks), tail_single and d0 + ND >= D)
        for si in range(len(splits) - 1):
            group = chunks[splits[si] : splits[si + 1]]
            pss = [
                psum2.tile([P, ND], f32, name=f"ps2_{gi}", tag=f"ps2_{gi}")
                for gi in range(len(group))
            ]
            for hc in range(HC):
                for gi, (hT, lc, _, _, _, _) in enumerate(group):
                    nc.tensor.matmul(
                        pss[gi][:],
                        lhsT=hT[:, hc, lc * P : (lc + 1) * P],
                        rhs=w2_tiles[hc][:],
                        start=(hc == 0),
                        stop=(hc == HC - 1),
                    )
            for gi, (hT, lc, y_ap, y_row0, cwt, cw_col) in enumerate(group):
                ot = opool.tile([P, ND], f32, name="ot", tag="ot")
                if cwt is not None:
                    nc.vector.tensor_scalar_mul(
                        ot[:], pss[gi][:], cwt[:, cw_col : cw_col + 1]
                    )
                else:
                    nc.vector.tensor_copy(ot[:], pss[gi][:])
                r0 = y_row0 + lc * P
                nc.sync.dma_start(out=y_ap[r0 : r0 + P, d0 : d0 + ND], in_=ot[:])


def _build(caps):
    import concourse.tile as tile
    from concourse import bacc, mybir

    f32 = mybir.dt.float32
    r32 = mybir.dt.float32r
    CR = sum(caps)

    nc = bacc.Bacc("TRN2", target_bir_lowering=False, debug=False,
                   num_devices=NCORES)

    # pre-tiled layouts (host packs partition-major for long-contiguous DMA):
    #   xr_t/xs_t: per x-half blocks [P, KC, ncols], flattened to [P, free]
    #   wr1/sw1:   per h-slice blocks [P, KC, hs], flattened to [P, KC*H]
    xr_t = nc.declare_dram_parameter("xr_t", [P, KC * CR], r32,
                                     isOutput=False)
    cw = nc.declare_dram_parameter("cw", [P, CR // P], f32, isOutput=False)
    wr1 = nc.declare_dram_parameter("wr1", [EPC, P, KC * H], r32,
                                    isOutput=False)
    wr2 = nc.declare_dram_parameter("wr2", [EPC, H, D], r32, isOutput=False)
    xs_t = nc.declare_dram_parameter("xs_t", [P, KC * TSH], r32, isOutput=False)
    sw1 = nc.declare_dram_parameter("sw1", [P, KC * H], r32, isOutput=False)
    sw2 = nc.declare_dram_parameter("sw2", [H, D], r32, isOutput=False)
    yr = nc.declare_dram_parameter("yr", [CR, D], f32, isOutput=True)
    ys = nc.declare_dram_parameter("ys", [TSH, D], f32, isOutput=True)

    with tile.TileContext(nc) as tc:
        with (
            tc.tile_pool(name="xpool", bufs=3) as xpool,
            tc.tile_pool(name="w1pool", bufs=2) as w1pool,
            tc.tile_pool(name="w2pool", bufs=13) as w2pool,
            tc.tile_pool(name="hpool", bufs=2) as hpool,
            tc.tile_pool(name="cwpool", bufs=2) as cwpool,
            tc.tile_pool(name="opool", bufs=3) as opool,
            tc.tile_pool(name="psum1", bufs=3, space="PSUM") as psum1,
            tc.tile_pool(name="psum2", bufs=1, space="PSUM") as psum2,
        ):
            pools = (xpool, w1pool, w2pool, hpool, cwpool, opool, psum1, psum2)

            def load_x(x_ap, col0, cap):
                halves, c0 = [], 0
                for ncols in _halves(cap):
                    xt = xpool.tile([P, KC, ncols], r32, name="xt", tag="xt")
                    off = KC * (col0 + c0)
                    nc.sync.dma_start(
                        out=xt[:],
                        in_=x_ap[:, off : off + KC * ncols].rearrange(
                            "p (kc t) -> p kc t", kc=KC
                        ),
                    )
                    halves.append((xt, c0, ncols))
                    c0 += ncols
                return halves

            # ---- shared expert: two MM1 sub-phases, one fused MM2 ----
            sh_chunks = []
            for q in range(TSH // SPH):
                hTs = _mm1_silu(
                    nc, mybir, pools,
                    lambda q=q: load_x(xs_t[:, :], q * SPH, SPH),
                    sw1[:, :], SPH,
                )
                sh_chunks += [
                    (hTs, lc, ys, q * SPH, None, 0) for lc in range(SPH // P)
                ]
            _mm2_out(nc, mybir, pools, sh_chunks, sw2[:, :])

            # ---- routed experts ----
            col0 = 0
            for j, cap in enumerate(caps):
                tch = cap // P
                ch0 = col0 // P
                cwt = cwpool.tile([P, tch], f32, name="cwt", tag="cwt")
                nc.sync.dma_start(out=cwt[:], in_=cw[:, ch0 : ch0 + tch])
                hTr = _mm1_silu(
                    nc, mybir, pools,
                    lambda j=j, col0=col0, cap=cap: load_x(
                        xr_t[:, :], col0, cap
                    ),
                    wr1[j], cap,
                )
                chunks = [(hTr, lc, yr, col0, cwt, lc) for lc in range(tch)]
                _mm2_out(nc, mybir, pools, chunks, wr2[j],
                         tail_single=(j == EPC - 1))
                col0 += cap
    nc.compile()
    return nc


def _get_nc(caps):
    caps = tuple(caps)
    nc = _NC_CACHE.get(caps)
    if nc is None:
        nc = _build(caps)
        _NC_CACHE[caps] = nc
    return nc


def kernel(x, gate_w, shared_w1, shared_w2, w1, w2, _trace=False):
    from concourse.bass_utils import run_bass_kernel_spmd

    x = np.asarray(x, dtype=np.float32)
    gate_w = np.asarray(gate_w, dtype=np.float32)
    shared_w1 = np.ascontiguousarray(np.asarray(shared_w1, dtype=np.float32))
    shared_w2 = np.ascontiguousarray(np.asarray(shared_w2, dtype=np.float32))
    w1 = np.ascontiguousarray(np.asarray(w1, dtype=np.float32))
    w2 = np.ascontiguousarray(np.asarray(w2, dtype=np.float32))

    xf = x.reshape(T, D)
    topk_w, topk_idx, aux = _gating(xf, gate_w)

    # group token slots by expert
    flat_e = topk_idx.reshape(-1)
    flat_w = topk_w.reshape(-1).astype(np.float32)
    flat_t = np.repeat(np.arange(T, dtype=np.int64), TOP_K)
    order = np.argsort(flat_e, kind="stable")
    counts = np.bincount(flat_e, minlength=E)
    starts = np.zeros(E + 1, dtype=np.int64)
    np.cumsum(counts, out=starts[1:])

    tok_by_e = [flat_t[order[starts[e] : starts[e + 1]]] for e in range(E)]
    w_by_e = [flat_w[order[starts[e] : starts[e + 1]]] for e in range(E)]

    # slot j of every core serves the j-th NCORES-sized group of experts,
    # ranked by count, so each slot's capacity fits its group max
    rank = np.argsort(-counts, kind="stable")
    assign = [[int(rank[j * NCORES + c]) for j in range(EPC)]
              for c in range(NCORES)]
    caps = tuple(
        max(P, int(-(-max(counts[rank[j * NCORES + c]]
                          for c in range(NCORES)) // P)) * P)
        for j in range(EPC)
    )
    CR = sum(caps)
    col0s = [sum(caps[:j]) for j in range(EPC)]

    xfT = np.ascontiguousarray(xf.T)  # [D, T]

    sw1_packed = [_pack_w1(shared_w1[s]) for s in range(NS)]
    wr1_packed = [_pack_w1(w1[e]) for e in range(E)]

    in_maps = []
    for c in range(NCORES):
        xr_cols = np.zeros((D, CR), dtype=np.float32)
        cwv = np.zeros(CR, dtype=np.float32)
        for j in range(EPC):
            e = assign[c][j]
            cnt = int(counts[e])
            xr_cols[:, col0s[j] : col0s[j] + cnt] = xfT[:, tok_by_e[e]]
            cwv[col0s[j] : col0s[j] + cnt] = w_by_e[e]
        s = c // (NCORES // NS)
        q = c % (NCORES // NS)
        in_maps.append({
            "xr_t": _pack_x(xr_cols, caps),
            "cw": np.ascontiguousarray(cwv.reshape(CR // P, P).T),
            "wr1": np.stack([wr1_packed[assign[c][j]] for j in range(EPC)]),
            "wr2": np.stack([w2[assign[c][j]] for j in range(EPC)]),
            "xs_t": _pack_x(
                xfT[:, q * TSH : (q + 1) * TSH], [SPH] * (TSH // SPH)
            ),
            "sw1": sw1_packed[s],
            "sw2": shared_w2[s],
        })

    nc = _get_nc(caps)
    res = run_bass_kernel_spmd(nc, in_maps, core_ids=list(range(NCORES)),
                               trace=_trace)
    if _trace:
        kernel.last_results = res

    # assemble: shared halves summed, then routed scatter-adds
    out = np.empty((T, D), dtype=np.float32)
    for q in range(NCORES // NS):
        out[q * TSH : (q + 1) * TSH] = res.results[q]["ys"]
    for q in range(NCORES // NS):
        out[q * TSH : (q + 1) * TSH] += res.results[NCORES // NS + q]["ys"]
    for c in range(NCORES):
        for j in range(EPC):
            e = assign[c][j]
            cnt = int(counts[e])
            out[tok_by_e[e]] += res.results[c]["yr"][
                col0s[j] : col0s[j] + cnt
            ]

    return out.reshape(B, S, D), aux


# revision 26
# speedup vs baseline: 1.0133x; 1.0126x over previous
"""DeepSeekMoE forward on 8 Trainium2 NeuronCores (Bass/Tile, expert-parallel).

Per core c (SPMD, one program, per-core data):
  - routed experts 2c, 2c+1: the host computes top-2 routing bit-exactly with
    the reference's jax ops (eager, CPU), gathers each expert's tokens, pads
    to a common capacity C, and ships them transposed [D, 2C].  The device
    runs the expert MLP (fp32r matmuls, SiLU on ScalarE) and scales rows by
    the combine weight on PSUM eviction.
  - shared experts: NS-split — cores 0-3 run shared expert 0, cores 4-7 run
    shared expert 1, each over 1024 tokens (two 512-token MM1 sub-phases
    sharing one MM2 pass so shared weights stream once).
Host assembly: sum the two shared halves, scatter-add routed rows (indices
are unique within one expert).
"""
import sys

for _p in ("/opt/trn_rl_repo", "/root/.axon_site/_ro/trn_rl_repo"):
    if _p not in sys.path:
        sys.path.insert(0, _p)

import numpy as np

P = 128
B, S, D, H, E, NS = 2, 2048, 2048, 1408, 16, 2
T = B * S
TOP_K = 2
NCORES = 8
EPC = E // NCORES          # routed experts per core
TSH = T * NS // NCORES     # shared-expert tokens per core
SPH = 512                  # shared tokens per MM1 sub-phase
KC = D // P                # contraction chunks (16)
HC = H // P                # hidden chunks (11)
ND = 512                   # MM2 free-dim (d) tile
H_SLICES = (384, 384, 384, 256)        # w1 streaming slices (sum = H)
MM2_GROUP = 5              # concurrent MM2 PSUM accumulators

_NC_CACHE = {}


def _gating(xf, gate_w):
    """Bit-exact replica of the reference's gating math, eager on CPU."""
    import jax
    import jax.numpy as jnp

    cpu = jax.devices("cpu")[0]
    with jax.default_device(cpu):
        xj = jnp.asarray(xf)
        gj = jnp.asarray(gate_w)
        logits = jnp.einsum("td,ed->te", xj, gj)
        probs = jax.nn.softmax(logits, axis=-1)
        topk_w, topk_idx = jax.lax.top_k(probs, TOP_K)
        topk_wn = topk_w / jnp.sum(topk_w, axis=-1, keepdims=True)
        density_1 = jnp.mean(probs, axis=0)
        density_1_proxy = jnp.mean(logits, axis=0)
        aux = jnp.sum(density_1 * density_1_proxy) * E
    return (
        np.asarray(topk_wn, dtype=np.float32),
        np.asarray(topk_idx),
        np.asarray(aux, dtype=np.float32),
    )


def _halves(cap):
    """Split cap into x-load halves (streams >=256 when able)."""
    if cap <= 256:
        return [cap]
    q = 64 if cap >= 512 else P
    h1 = -(-cap // 2 // q) * q
    return [h1, cap - h1]


def _pack_x(mat, caps):
    """[D, total] -> [P, KC*total]: per-half blocks [P, KC, ncols], flattened.

    caps: list of span widths; each span is packed per its _halves split.
    """
    total = mat.shape[1]
    out = np.empty((P, KC * total), dtype=np.float32)
    c0 = 0
    for cap in caps:
        for ncols in _halves(cap):
            blk = (
                mat[:, c0 : c0 + ncols]
                .reshape(KC, P, ncols)
                .transpose(1, 0, 2)
                .reshape(P, KC * ncols)
            )
            out[:, KC * c0 : KC * (c0 + ncols)] = blk
            c0 += ncols
    assert c0 == total
    return out


def _pack_w1(w):
    """[D, H] -> [P, KC*H]: per h-slice blocks [P, KC, hs], flattened."""
    out = np.empty((P, KC * H), dtype=np.float32)
    h0 = 0
    for hs in H_SLICES:
        blk = (
            w[:, h0 : h0 + hs]
            .reshape(KC, P, hs)
            .transpose(1, 0, 2)
            .reshape(P, KC * hs)
        )
        out[:, KC * h0 : KC * (h0 + hs)] = blk
        h0 += hs
    assert h0 == H
    return out


def _load_w1_slice(nc, w1pool, r32, w1_ap, h0, hs):
    w1t = w1pool.tile([P, KC, hs], r32, name="w1t", tag="w1t")
    nc.sync.dma_start(
        out=w1t[:],
        in_=w1_ap[:, KC * h0 : KC * (h0 + hs)].rearrange(
            "p (kc h) -> p kc h", kc=KC
        ),
    )
    return w1t


def _mm1_silu(nc, mybir, pools, x_loader, w1_ap, cap):
    """hT[h, t] = silu(sum_d w1[d, h] x[d, t]) for `cap` tokens."""
    f32 = mybir.dt.float32
    r32 = mybir.dt.float32r
    (xpool, w1pool, w2pool, hpool, cwpool, opool, psum1, psum2) = pools

    # first w1 slice before the x tiles: both gate the first matmul
    w1t = _load_w1_slice(nc, w1pool, r32, w1_ap, 0, H_SLICES[0])
    x_halves = x_loader()

    # hT spans `cap` columns; only the loader-provided columns are written
    hT = hpool.tile([P, HC, cap], r32, name="hT", tag="hT")
    h0 = 0
    for si, hs in enumerate(H_SLICES):
        if si > 0:
            w1t = _load_w1_slice(nc, w1pool, r32, w1_ap, h0, hs)
        for hci in range(hs // P):
            hc = h0 // P + hci
            for (xt, c0, ncols) in x_halves:
                ps = psum1.tile([P, ncols], f32, name="ps1", tag="ps1")
                for kc in range(KC):
                    nc.tensor.matmul(
                        ps[:],
                        lhsT=w1t[:, kc, hci * P : (hci + 1) * P],
                        rhs=xt[:, kc, 0:ncols],
                        start=(kc == 0),
                        stop=(kc == KC - 1),
                    )
                nc.scalar.activation(
                    hT[:, hc, c0 : c0 + ncols], ps[:],
                    mybir.ActivationFunctionType.Silu,
                )
        h0 += hs
    assert h0 == H
    return hT


def _group_splits(n, tail_single):
    splits = list(range(0, n, MM2_GROUP))
    if tail_single and n > 1 and (n - splits[-1]) > 1:
        splits.append(n - 1)
    return splits + [n]


def _mm2_out(nc, mybir, pools, chunks, w2_ap, tail_single=False):
    """y[t, d] = [cw[t] *] sum_h hT[h, t] w2[h, d].

    chunks: list of (hT, local_chunk, y_ap, y_row0, cwt_or_None, cw_col);
    streams w2 once per d0 across all chunks.
    """
    f32 = mybir.dt.float32
    r32 = mybir.dt.float32r
    (xpool, w1pool, w2pool, hpool, cwpool, opool, psum1, psum2) = pools

    for d0 in range(0, D, ND):
        w2_tiles = []
        for hc in range(HC):
            w2t = w2pool.tile([P, ND], r32, name="w2t", tag="w2t")
            nc.sync.dma_start(
                out=w2t[:], in_=w2_ap[hc * P : (hc + 1) * P, d0 : d0 + ND]
            )
            w2_tiles.append(w2t)
        splits = _group_splits(len(chunks), tail_single and d0 + ND >= D)
        for si in range(len(splits) - 1):
            group = chunks[splits[si] : splits[si + 1]]
            pss = [
                psum2.tile([P, ND], f32, name=f"ps2_{gi}", tag=f"ps2_{gi}")
                for gi in range(len(group))
            ]
            for hc in range(HC):
                for gi, (hT, lc, _, _, _, _) in enumerate(group):
                    nc.tensor.matmul(
                        pss[gi][:],
                        lhsT=hT[:, hc, lc * P : (lc + 1) * P],
                        rhs=w2_tiles[hc][:],
                        start=(hc == 0),
                        stop=(hc == HC - 1),
                    )
            for gi, (hT, lc, y_ap, y_row0, cwt, cw_col) in enumerate(group):
                ot = opool.tile([P, ND], f32, name="ot", tag="ot")
                if cwt is not None:
                    nc.vector.tensor_scalar_mul(
                        ot[:], pss[gi][:], cwt[:, cw_col : cw_col + 1]
                    )
                else:
                    nc.vector.tensor_copy(ot[:], pss[gi][:])
                r0 = y_row0 + lc * P
                nc.sync.dma_start(out=y_ap[r0 : r0 + P, d0 : d0 + ND], in_=ot[:])


def _build(key):
    import concourse.tile as tile
    from concourse import bacc, mybir

    caps, widths = key
    f32 = mybir.dt.float32
    r32 = mybir.dt.float32r
    CR = sum(caps)
    XW = sum(widths)

    nc = bacc.Bacc("TRN2", target_bir_lowering=False, debug=False,
                   num_devices=NCORES)

    # pre-tiled layouts (host packs partition-major for long-contiguous DMA):
    #   xr_t/xs_t: per x-half blocks [P, KC, ncols], flattened to [P, free]
    #   wr1/sw1:   per h-slice blocks [P, KC, hs], flattened to [P, KC*H]
    xr_t = nc.declare_dram_parameter("xr_t", [P, KC * XW], r32,
                                     isOutput=False)
    cw = nc.declare_dram_parameter("cw", [P, CR // P], f32, isOutput=False)
    wr1 = nc.declare_dram_parameter("wr1", [EPC, P, KC * H], r32,
                                    isOutput=False)
    wr2 = nc.declare_dram_parameter("wr2", [EPC, H, D], r32, isOutput=False)
    xs_t = nc.declare_dram_parameter("xs_t", [P, KC * TSH], r32, isOutput=False)
    sw1 = nc.declare_dram_parameter("sw1", [P, KC * H], r32, isOutput=False)
    sw2 = nc.declare_dram_parameter("sw2", [H, D], r32, isOutput=False)
    yr = nc.declare_dram_parameter("yr", [CR, D], f32, isOutput=True)
    ys = nc.declare_dram_parameter("ys", [TSH, D], f32, isOutput=True)

    with tile.TileContext(nc) as tc:
        with (
            tc.tile_pool(name="xpool", bufs=3) as xpool,
            tc.tile_pool(name="w1pool", bufs=2) as w1pool,
            tc.tile_pool(name="w2pool", bufs=13) as w2pool,
            tc.tile_pool(name="hpool", bufs=2) as hpool,
            tc.tile_pool(name="cwpool", bufs=2) as cwpool,
            tc.tile_pool(name="opool", bufs=3) as opool,
            tc.tile_pool(name="psum1", bufs=3, space="PSUM") as psum1,
            tc.tile_pool(name="psum2", bufs=1, space="PSUM") as psum2,
        ):
            pools = (xpool, w1pool, w2pool, hpool, cwpool, opool, psum1, psum2)

            def load_x(x_ap, col0, cap):
                halves, c0 = [], 0
                for ncols in _halves(cap):
                    xt = xpool.tile([P, KC, ncols], r32, name="xt", tag="xt")
                    off = KC * (col0 + c0)
                    nc.sync.dma_start(
                        out=xt[:],
                        in_=x_ap[:, off : off + KC * ncols].rearrange(
                            "p (kc t) -> p kc t", kc=KC
                        ),
                    )
                    halves.append((xt, c0, ncols))
                    c0 += ncols
                return halves

            # ---- shared expert: two MM1 sub-phases, one fused MM2 ----
            sh_chunks = []
            for q in range(TSH // SPH):
                hTs = _mm1_silu(
                    nc, mybir, pools,
                    lambda q=q: load_x(xs_t[:, :], q * SPH, SPH),
                    sw1[:, :], SPH,
                )
                sh_chunks += [
                    (hTs, lc, ys, q * SPH, None, 0) for lc in range(SPH // P)
                ]
            _mm2_out(nc, mybir, pools, sh_chunks, sw2[:, :])

            # ---- routed experts ----
            # MM1 computes only `width` token columns (covers every expert's
            # actual count); MM2/cw/yr keep the 128-aligned `cap` layout.
            # hT columns in [width, cap) are never written — they feed output
            # rows past each expert's count, which the host never reads.
            col0 = 0
            wcol0 = 0
            for j, (cap, width) in enumerate(zip(caps, widths)):
                tch = cap // P
                ch0 = col0 // P
                cwt = cwpool.tile([P, tch], f32, name="cwt", tag="cwt")
                nc.sync.dma_start(out=cwt[:], in_=cw[:, ch0 : ch0 + tch])
                hTr = _mm1_silu(
                    nc, mybir, pools,
                    lambda wcol0=wcol0, width=width: load_x(
                        xr_t[:, :], wcol0, width
                    ),
                    wr1[j], cap,
                )
                chunks = [(hTr, lc, yr, col0, cwt, lc) for lc in range(tch)]
                _mm2_out(nc, mybir, pools, chunks, wr2[j],
                         tail_single=(j == EPC - 1))
                col0 += cap
                wcol0 += width
    nc.compile()
    return nc


def _get_nc(caps, widths):
    key = (tuple(caps), tuple(widths))
    nc = _NC_CACHE.get(key)
    if nc is None:
        nc = _build(key)
        _NC_CACHE[key] = nc
    return nc


def kernel(x, gate_w, shared_w1, shared_w2, w1, w2, _trace=False):
    from concourse.bass_utils import run_bass_kernel_spmd

    x = np.asarray(x, dtype=np.float32)
    gate_w = np.asarray(gate_w, dtype=np.float32)
    shared_w1 = np.ascontiguousarray(np.asarray(shared_w1, dtype=np.float32))
    shared_w2 = np.ascontiguousarray(np.asarray(shared_w2, dtype=np.float32))
    w1 = np.ascontiguousarray(np.asarray(w1, dtype=np.float32))
    w2 = np.ascontiguousarray(np.asarray(w2, dtype=np.float32))

    xf = x.reshape(T, D)
    topk_w, topk_idx, aux = _gating(xf, gate_w)

    # group token slots by expert
    flat_e = topk_idx.reshape(-1)
    flat_w = topk_w.reshape(-1).astype(np.float32)
    flat_t = np.repeat(np.arange(T, dtype=np.int64), TOP_K)
    order = np.argsort(flat_e, kind="stable")
    counts = np.bincount(flat_e, minlength=E)
    starts = np.zeros(E + 1, dtype=np.int64)
    np.cumsum(counts, out=starts[1:])

    tok_by_e = [flat_t[order[starts[e] : starts[e + 1]]] for e in range(E)]
    w_by_e = [flat_w[order[starts[e] : starts[e + 1]]] for e in range(E)]

    # slot j of every core serves the j-th NCORES-sized group of experts,
    # ranked by count, so each slot's capacity fits its group max
    rank = np.argsort(-counts, kind="stable")
    assign = [[int(rank[j * NCORES + c]) for j in range(EPC)]
              for c in range(NCORES)]
    caps = tuple(
        max(P, int(-(-max(counts[rank[j * NCORES + c]]
                          for c in range(NCORES)) // P)) * P)
        for j in range(EPC)
    )
    CR = sum(caps)
    col0s = [sum(caps[:j]) for j in range(EPC)]
    # MM1 width per slot: group max rounded to 64, min 512 (keeps x-half
    # streams >=256 for full-rate fp32r), never above cap
    gmax = [max(int(counts[rank[j * NCORES + c]]) for c in range(NCORES))
            for j in range(EPC)]
    widths = tuple(
        min(caps[j], max(512, -(-gmax[j] // 64) * 64)) for j in range(EPC)
    )
    wcol0s = [sum(widths[:j]) for j in range(EPC)]
    XW = sum(widths)

    xfT = np.ascontiguousarray(xf.T)  # [D, T]

    sw1_packed = [_pack_w1(shared_w1[s]) for s in range(NS)]
    wr1_packed = [_pack_w1(w1[e]) for e in range(E)]

    in_maps = []
    for c in range(NCORES):
        xr_cols = np.zeros((D, XW), dtype=np.float32)
        cwv = np.zeros(CR, dtype=np.float32)
        for j in range(EPC):
            e = assign[c][j]
            cnt = int(counts[e])
            xr_cols[:, wcol0s[j] : wcol0s[j] + cnt] = xfT[:, tok_by_e[e]]
            cwv[col0s[j] : col0s[j] + cnt] = w_by_e[e]
        s = c // (NCORES // NS)
        q = c % (NCORES // NS)
        in_maps.append({
            "xr_t": _pack_x(xr_cols, widths),
            "cw": np.ascontiguousarray(cwv.reshape(CR // P, P).T),
            "wr1": np.stack([wr1_packed[assign[c][j]] for j in range(EPC)]),
            "wr2": np.stack([w2[assign[c][j]] for j in range(EPC)]),
            "xs_t": _pack_x(
                xfT[:, q * TSH : (q + 1) * TSH], [SPH] * (TSH // SPH)
            ),
            "sw1": sw1_packed[s],
            "sw2": shared_w2[s],
        })

    nc = _get_nc(caps, widths)
    res = run_bass_kernel_spmd(nc, in_maps, core_ids=list(range(NCORES)),
                               trace=_trace)
    if _trace:
        kernel.last_results = res

    # assemble: shared halves summed, then routed scatter-adds
    out = np.empty((T, D), dtype=np.float32)
    for q in range(NCORES // NS):
        out[q * TSH : (q + 1) * TSH] = res.results[q]["ys"]
    for q in range(NCORES // NS):
        out[q * TSH : (q + 1) * TSH] += res.results[NCORES // NS + q]["ys"]
    for c in range(NCORES):
        for j in range(EPC):
            e = assign[c][j]
            cnt = int(counts[e])
            out[tok_by_e[e]] += res.results[c]["yr"][
                col0s[j] : col0s[j] + cnt
            ]

    return out.reshape(B, S, D), aux
